# revision 41
# baseline (speedup 1.0000x reference)
"""MultiHeadSelfAttention + residual + LayerNorm on 8 TRN2 NeuronCores.

Sharding: 2 cores per batch element (B=4), heads split 8/8 within the pair
(tensor parallel). Each core: QKV for its heads over the full sequence
(bf16 matmuls), attention, output projection partial, ONE pairwise bf16
AllReduce per 1024-row half (the first fires mid-attention where partner
skew is tiny), residual + LayerNorm.

Attention core: scores for a kti PAIR land in one 2-bank PSUM tile so a
single wide exp (1024 elems/instr) amortizes the ACT access bubble; exp
emits fp8e4 probabilities directly (bias -2 keeps exp under e4m3's 448
NaN-saturation; it cancels in the softmax ratio); PV runs as fp8
DoubleRow matmuls (K=256 per instruction, 2x stream rate) against an
fp8 [V|ones|pad] stationary padded to 68 columns (odd 65 fails the DR
ldweights ISA check). The ones column folds the softmax denominator into
the PV matmul. Scores are emitted BEFORE pv(kti-1) so the in-order PE
queue never head-of-line blocks on ACT. LayerNorm: sqrt batched per half
([128,8] one ACT call, avoids Exp<->Sqrt table thrash); LN(0) emitted at
qc=1 j=2 when its AR is long done so it never stalls the DVE queue.

Softmax denominators are broadcast across partitions via a DRAM bounce
(SBUF-source partition-broadcast DMA is illegal, and gpsimd
partition_broadcast would serialize behind the blocking collectives on the
gpsimd queue). Reciprocals run on a [64, 8] scatter so all DVE lanes work.
Known-bad variants: fp8 DoubleRow for QKV (-regression, DR overhead beats
the stream saving in accumulation chains), gpsimd PSUM reads (illegal),
DVE pow (no ISA support), single-partition reciprocal (one lane, 5us).

Self-contained: shapes/sharding hardcoded; builds and caches the NEFF on
first call. The host slices each core's owned 1024-row half.
"""
import numpy as np
import ml_dtypes

import concourse.bass as bass
import concourse.tile as tile
from concourse import bacc, mybir
from concourse.bass_utils import run_bass_kernel_spmd
from concourse.masks import make_identity

F32 = mybir.dt.float32
F32R = mybir.dt.float32r
BF16 = mybir.dt.bfloat16
FP8E4 = mybir.dt.float8e4

B, S, D, H, DEPTH = 4, 2048, 1024, 16, 64
HL = 8            # heads per core
EL = 512          # local e width (HL * DEPTH)
CT = 8            # c tiles (D / 128)
ST = 16           # s tiles (S / 128)
SC = 4            # s chunks (S / 512)
ET = 4            # local e tiles (EL / 128)
EPS = 1e-6
PDE = 68          # padded V stationary width (64 depth + ones + pad, 4-aligned)
RG = [[0, 1], [2, 3], [4, 5], [6, 7]]
NCH = 4           # output row chunks (S / 512) for proj+AR+LN pipeline

_CACHE = {}
_LAST_IN_MAPS = None


def _build(fast_ln=False):
    nc = bacc.Bacc("TRN2", target_bir_lowering=False, debug=False, num_devices=8)

    x_in = nc.dram_tensor("x", [S, D], F32, kind="ExternalInput")
    xb_in = nc.dram_tensor("xb", [S, D], BF16, kind="ExternalInput")
    wq_in = nc.dram_tensor("wq", [D, EL], BF16, kind="ExternalInput")
    wk_in = nc.dram_tensor("wk", [D, EL], BF16, kind="ExternalInput")
    wv_in = nc.dram_tensor("wv", [D, EL], BF16, kind="ExternalInput")
    wo_in = nc.dram_tensor("wo", [EL, D], BF16, kind="ExternalInput")
    bqk_in = nc.dram_tensor("bqk", [128, 2 * ET], F32, kind="ExternalInput")
    bv_in = nc.dram_tensor("bv", [1, EL], F32, kind="ExternalInput")
    bo_in = nc.dram_tensor("bo", [1, D], F32, kind="ExternalInput")
    gamma_in = nc.dram_tensor("gamma", [1, D], F32, kind="ExternalInput")
    beta_in = nc.dram_tensor("beta", [1, D], F32, kind="ExternalInput")
    y_out = nc.dram_tensor("y", [S, D], F32, kind="ExternalOutput")

    with tile.TileContext(nc) as tc:
        with tc.tile_pool(name="const", bufs=1) as const, \
             tc.tile_pool(name="dram", bufs=1, space="DRAM") as dram:

            ident_f = const.tile([128, 128], F32)
            make_identity(nc, ident_f[:])
            ident = const.tile([128, 128], BF16)
            nc.vector.tensor_copy(ident[:], ident_f[:])
            ones1 = const.tile([128, 1], FP8E4)
            nc.gpsimd.memset(ones1[:], 1.0)
            eps_sb = const.tile([128, 1], F32)
            nc.gpsimd.memset(eps_sb[:], EPS)
            nbias = const.tile([128, 1], F32)
            nc.gpsimd.memset(nbias[:], -2.0)

            bqk_sb = const.tile([128, 2 * ET], F32)
            nc.sync.dma_start(bqk_sb[:], bqk_in.ap()[:])
            bv_bc = const.tile([128, EL], F32)
            nc.sync.dma_start(bv_bc[:], bv_in.ap().to_broadcast((128, EL)))

            y_part = [dram.tile([1024, D], BF16, name=f"y_part{i}") for i in range(2)]
            ar_out = [dram.tile([1024, D], BF16, name=f"ar_out{i}") for i in range(2)]
            den_d = dram.tile([2 * ET * 2 * 2, 1, 512], F32)
            rec_d = dram.tile([2 * ET * 2 * 2, 64, 8], F32)

            with tc.tile_pool(name="qkv", bufs=1) as qkvp:
                kt = qkvp.tile([128, ET, S], BF16)                 # K^T [e, s]
                qt = qkvp.tile([128, ET, S], BF16)                 # Q^T [e, s]
                vt = qkvp.tile([128, ST, HL, PDE], FP8E4)          # V + ones + pad
                nc.vector.tensor_copy(vt[:, :, :, DEPTH:PDE],
                                      ones1[:].to_broadcast((128, ST, HL, PDE - DEPTH)))

                # ---- phase A: transpose X per chunk; project Q, K, V ----
                with tc.tile_pool(name="xnA", bufs=3) as xnA, \
                     tc.tile_pool(name="xtA", bufs=2) as xtA, \
                     tc.tile_pool(name="w3", bufs=1) as w3, \
                     tc.tile_pool(name="tpA", bufs=4, space="PSUM") as tpA, \
                     tc.tile_pool(name="psA", bufs=4, space="PSUM") as psA:
                    wsb = {}
                    for nm, wdram in (("q", wq_in), ("k", wk_in), ("v", wv_in)):
                        wsb[nm] = w3.tile([128, CT, EL], BF16, name=f"w{nm}")
                        for ci in range(CT):
                            # scalar-queue DMA: keeps the weight prefetch off
                            # the sync queues so the first x tiles (which gate
                            # the transposes) aren't stuck behind 3MB of W
                            nc.scalar.dma_start(wsb[nm][:, ci, :],
                                                wdram.ap()[128 * ci:128 * (ci + 1), :])
                    for sc in range(SC):
                        cs = slice(512 * sc, 512 * (sc + 1))
                        xt_c = xtA.tile([128, CT, 512], BF16, name="xt_c", tag="xt_c")
                        for sl in range(4):
                            si = 4 * sc + sl
                            xn = xnA.tile([128, D], BF16, name="xn", tag="xn")
                            nc.sync.dma_start(xn[:], xb_in.ap()[128 * si:128 * (si + 1), :])
                            for ci in range(CT):
                                tp = tpA.tile([128, 128], BF16, name="tp", tag="tp")
                                nc.tensor.transpose(tp[:], xn[:, 128 * ci:128 * (ci + 1)],
                                                    ident[:])
                                nc.vector.tensor_copy(xt_c[:, ci, 128 * sl:128 * (sl + 1)],
                                                      tp[:])
                        for dst, wname, bcol in ((qt, "q", 0), (kt, "k", ET)):
                            for j in range(ET):
                                ps = psA.tile([128, 512], F32, name="pqk", tag="pqk")
                                for ci in range(CT):
                                    nc.tensor.matmul(
                                        ps[:], wsb[wname][:, ci, 128 * j:128 * (j + 1)],
                                        xt_c[:, ci, :], start=(ci == 0), stop=(ci == CT - 1))
                                nc.vector.tensor_scalar_add(
                                    dst[:, j, cs], ps[:], bqk_sb[:, bcol + j:bcol + j + 1])
                        for sl in range(4):
                            si = 4 * sc + sl
                            ps = psA.tile([128, 512], F32, name="pv", tag="pqk")
                            for ci in range(CT):
                                nc.tensor.matmul(
                                    ps[:], xt_c[:, ci, 128 * sl:128 * (sl + 1)],
                                    wsb["v"][:, ci, :], start=(ci == 0), stop=(ci == CT - 1))
                            nc.vector.tensor_add(
                                vt[:, si, :, 0:DEPTH],
                                ps[:].rearrange("p (h e) -> p h e", h=HL),
                                bv_bc[:].rearrange("p (h e) -> p h e", h=HL))

                # ---- phase B: attention per q-chunk + chunk projection + AR + LN ----
                with tc.tile_pool(name="wo", bufs=1) as wop, \
                     tc.tile_pool(name="atc", bufs=1) as atcp, \
                     tc.tile_pool(name="ep3", bufs=2) as ep3, \
                     tc.tile_pool(name="psb", bufs=2) as psb, \
                     tc.tile_pool(name="ysb", bufs=2) as ysb, \
                     tc.tile_pool(name="lnc", bufs=1) as lnc, \
                     tc.tile_pool(name="ln", bufs=3) as ln, \
                     tc.tile_pool(name="sps", bufs=2, space="PSUM") as sps, \
                     tc.tile_pool(name="aps", bufs=1, space="PSUM") as aps:
                    wo_sb = wop.tile([128, ET, D], BF16)
                    for j in range(ET):
                        nc.sync.dma_start(wo_sb[:, j, :], wo_in.ap()[128 * j:128 * (j + 1), :])
                    bo_bc = lnc.tile([128, D], F32)
                    nc.sync.dma_start(bo_bc[:], bo_in.ap().to_broadcast((128, D)))
                    gam_bc = lnc.tile([128, D], F32)
                    nc.sync.dma_start(gam_bc[:], gamma_in.ap().to_broadcast((128, D)))
                    bet_bc = lnc.tile([128, D], F32)
                    nc.sync.dma_start(bet_bc[:], beta_in.ap().to_broadcast((128, D)))

                    def emit_proj_ar(h):
                        # output projection for rows [1024h, 1024h+1024) into a
                        # merged half AllReduce; proj psums reuse the score
                        # pool's slots (4 deep) to hide the y_sb add latency
                        a_t = a_ts[h]
                        for stl in range(8):
                            ss = slice(128 * stl, 128 * (stl + 1))
                            for mh in range(2):
                                ms = slice(512 * mh, 512 * (mh + 1))
                                ps = aps.tile([128, 512], F32, name="py",
                                              tag=f"acc{stl % 2}")
                                for j in range(ET):
                                    nc.tensor.matmul(ps[:], a_t[:, j, ss], wo_sb[:, j, ms],
                                                     start=(j == 0), stop=(j == ET - 1))
                                # bo/2 folded here (host passes bo/2; the
                                # pairwise AllReduce sums it back to bo)
                                y_sb = ysb.tile([128, 512], BF16, name="y_sb", tag="y_sb")
                                nc.vector.tensor_add(y_sb[:], ps[:], bo_bc[:, ms])
                                nc.sync.dma_start(y_part[h][ss, ms], y_sb[:])
                        nc.gpsimd.collective_compute(
                            "AllReduce", mybir.AluOpType.add,
                            replica_groups=RG,
                            ins=[y_part[h].opt()], outs=[ar_out[h].opt()])

                    def emit_ln(h):
                        # residual + LN for rows [1024h, ...); batched
                        # pow(var+eps, 0.5) on DVE — no ACT table thrash
                        tts = []
                        mvp = ln.tile([128, 8, 2], F32, name="mvp", tag="mvp")
                        for rt in range(8):
                            rs = slice(128 * rt, 128 * (rt + 1))
                            grow = slice(1024 * h + 128 * rt, 1024 * h + 128 * (rt + 1))
                            t_b = ln.tile([128, D], BF16, name="t_b", tag="t_b")
                            nc.sync.dma_start(t_b[:], ar_out[h][rs, :])
                            t = ln.tile([128, D], F32, name="t", tag="t", bufs=9)
                            nc.sync.dma_start(t[:], x_in.ap()[grow, :])
                            nc.vector.tensor_add(t[:], t[:], t_b[:])
                            stats = ln.tile([128, 2, 6], F32, name="stats", tag="stats")
                            tv = t[:].rearrange("p (a b) -> p a b", a=2)
                            for sub in range(2):
                                nc.vector.bn_stats(stats[:, sub, :], tv[:, sub, :])
                            nc.vector.bn_aggr(mvp[:, rt, :], stats[:])
                            tts.append((t, grow))
                        # rsqrt(var+eps) fully on DVE (magic-constant seed +
                        # 2 Newton steps) so LN never touches the ACT engine:
                        # an ACT Sqrt here would head-of-line block later
                        # exps behind the LN stats chain and thrash the
                        # Exp<->Sqrt activation tables
                        stdp = ln.tile([128, 8], F32, name="stdp", tag="stdp")
                        nc.vector.tensor_scalar(stdp[:], mvp[:, :, 1], EPS, None,
                                                mybir.AluOpType.add)
                        sti = stdp[:].bitcast(mybir.dt.int32)
                        nc.vector.tensor_scalar(sti, sti, 1, None,
                                                mybir.AluOpType.logical_shift_right)
                        nc.vector.tensor_scalar(sti, sti, -1, 0x5F3759DF,
                                                mybir.AluOpType.mult,
                                                mybir.AluOpType.add)
                        vhalf = ln.tile([128, 8], F32, name="vhalf", tag="vhalf")
                        nc.vector.tensor_scalar(vhalf[:], mvp[:, :, 1], EPS, -0.5,
                                                mybir.AluOpType.add,
                                                mybir.AluOpType.mult)
                        for _ in range(2):
                            yy = ln.tile([128, 8], F32, name="yy", tag="yy")
                            nc.vector.tensor_mul(yy[:], stdp[:], stdp[:])
                            nc.vector.tensor_mul(yy[:], yy[:], vhalf[:])
                            nc.vector.tensor_scalar(yy[:], yy[:], 1.5, None,
                                                    mybir.AluOpType.add)
                            nc.vector.tensor_mul(stdp[:], stdp[:], yy[:])
                        for rt, (t, grow) in enumerate(tts):
                            o = ln.tile([128, D], F32, name="o", tag="o")
                            nc.vector.tensor_scalar(
                                o[:], t[:], mvp[:, rt, 0:1], stdp[:, rt:rt + 1],
                                mybir.AluOpType.subtract, mybir.AluOpType.mult)
                            if not fast_ln:
                                nc.vector.tensor_mul(o[:], o[:], gam_bc[:])
                                nc.vector.tensor_add(o[:], o[:], bet_bc[:])
                            nc.sync.dma_start(y_out.ap()[grow, :], o[:])

                    a_ts = []
                    for qc in range(2):
                        a_t = atcp.tile([128, ET, 1024], BF16, name=f"a_t{qc}",
                                        tag=f"a_t{qc}")
                        a_ts.append(a_t)
                        for j in range(ET):
                            for half in range(2):
                                qs = slice(1024 * qc + 512 * half,
                                           1024 * qc + 512 * (half + 1))
                                asl = slice(512 * half, 512 * (half + 1))
                                accs = [aps.tile([PDE, 512], F32,
                                                 name=f"acc{h}", tag=f"acc{h}")
                                        for h in range(2)]

                                def emit_pv_pair(t, pp):
                                    # fp8 DoubleRow: two k-tiles (K=256) per
                                    # matmul at 2x stream rate
                                    for h01 in range(2):
                                        nc.tensor.matmul(
                                            accs[h01][:],
                                            vt[:, 2 * t:2 * t + 2, 2 * j + h01, :],
                                            pp[h01][:],
                                            start=(t == 0), stop=(t == ST // 2 - 1),
                                            perf_mode=mybir.MatmulPerfMode.DoubleRow)

                                p_prev = None
                                for t in range(ST // 2):
                                    # scores for a kti PAIR land in one 2-bank
                                    # PSUM tile so ONE wide exp (1024 elems)
                                    # amortizes the ACT access bubble
                                    s_w = {}
                                    for h01 in range(2):
                                        s_w[h01] = sps.tile(
                                            [128, 2, 512], F32, name=f"sw{h01}",
                                            tag=f"sw{h01}",
                                            bufs=(2 if h01 == 0 else 1))
                                    for sub in range(2):
                                        kti = 2 * t + sub
                                        ks = slice(128 * kti, 128 * (kti + 1))
                                        for h01 in range(2):
                                            rows = slice(64 * h01, 64 * (h01 + 1))
                                            nc.tensor.matmul(s_w[h01][:, sub, :],
                                                             kt[rows, j, ks],
                                                             qt[rows, j, qs],
                                                             start=True, stop=True)
                                    if p_prev is not None:
                                        emit_pv_pair(t - 1, p_prev)
                                    p_cur = {}
                                    for h01 in range(2):
                                        p = psb.tile([128, 2, 512], FP8E4,
                                                     name=f"pp{h01}", tag=f"pp{h01}")
                                        # bias -2 shifts exp into fp8e4 range
                                        # (e4m3 saturates to NaN above 448);
                                        # cancels exactly in the softmax ratio
                                        nc.scalar.activation(
                                            p[:], s_w[h01][:],
                                            mybir.ActivationFunctionType.Exp,
                                            scale=0.125, bias=nbias[:])
                                        p_cur[h01] = p
                                    p_prev = p_cur
                                emit_pv_pair(ST // 2 - 1, p_prev)
                                for h01 in range(2):
                                    idx = ((qc * ET + j) * 2 + half) * 2 + h01
                                    acc_sb = ep3.tile([DEPTH + 1, 512], F32,
                                                      name="acc_sb", tag="acc_sb")
                                    nc.vector.tensor_copy(acc_sb[:], accs[h01][0:DEPTH + 1, :])
                                    nc.sync.dma_start(den_d[idx],
                                                      acc_sb[DEPTH:DEPTH + 1, :])
                                    rin = ep3.tile([64, 8], F32, name="rin", tag="rin")
                                    nc.sync.dma_start(rin[:], den_d[idx].rearrange(
                                        "a (p f) -> (a p) f", p=64))
                                    nc.vector.reciprocal(rin[:], rin[:])
                                    nc.sync.dma_start(rec_d[idx], rin[:])
                                    rbc = ep3.tile([64, 512], F32, name="rbc", tag="rbc")
                                    rsrc = rec_d[idx]
                                    nc.sync.dma_start(
                                        rbc[:],
                                        bass.AP(tensor=rsrc.tensor, offset=rsrc.offset,
                                                ap=[[0, 64], [1, 512]]))
                                    if h01 == 0:
                                        nc.vector.tensor_mul(a_t[0:64, j, asl],
                                                             acc_sb[0:DEPTH, :], rbc[:])
                                    else:
                                        nrm = ep3.tile([64, 512], BF16, name="nrm", tag="nrm")
                                        nc.vector.tensor_mul(nrm[:], acc_sb[0:DEPTH, :], rbc[:])
                                        nc.sync.dma_start(a_t[64:128, j, asl], nrm[:])
                            if qc == 1 and j == 1:
                                # first half's proj + AR fire here: partner
                                # skew is tiny, a_t0's trailing nrm DMAs are
                                # long done (no PE wait), and the AR completes
                                # while attention j=2..3 still runs
                                emit_proj_ar(0)
                            if qc == 1 and j == 2:
                                # LN(0): AR0 finished long ago, so no DVE
                                # wait; its DVE work hides inside j=3's
                                # PE/ACT window
                                emit_ln(0)
                    emit_proj_ar(1)
                    emit_ln(1)

    nc.compile()
    return nc


def kernel(inputs, Wq, bq, Wk, bk, Wv, bv, Wo, bo, gamma, beta):
    fast_ln = bool(np.allclose(np.asarray(gamma), 1.0)
                   and np.allclose(np.asarray(beta), 0.0))
    key = ("nc", fast_ln)
    if key not in _CACHE:
        _CACHE[key] = _build(fast_ln)
    nc = _CACHE[key]

    inputs = np.ascontiguousarray(np.asarray(inputs, dtype=np.float32))
    Wq = np.asarray(Wq, np.float32); Wk = np.asarray(Wk, np.float32)
    Wv = np.asarray(Wv, np.float32); Wo = np.asarray(Wo, np.float32)
    bq = np.asarray(bq, np.float32); bk = np.asarray(bk, np.float32)
    bv = np.asarray(bv, np.float32); bo = np.asarray(bo, np.float32)
    gamma = np.asarray(gamma, np.float32); beta = np.asarray(beta, np.float32)
    BF = ml_dtypes.bfloat16
    F8 = mybir.dt.np(FP8E4)

    in_maps = []
    for c in range(8):
        b, hf = c // 2, c % 2
        es = slice(EL * hf, EL * (hf + 1))
        bqk = np.concatenate([bq[es].reshape(ET, 128).T, bk[es].reshape(ET, 128).T],
                             axis=1)
        in_maps.append({
            "x": inputs[b],
            "xb": np.ascontiguousarray(inputs[b].astype(BF)),
            "wq": np.ascontiguousarray(Wq[:, es].astype(BF)),
            "wk": np.ascontiguousarray(Wk[:, es].astype(BF)),
            "wv": np.ascontiguousarray(Wv[:, es].astype(BF)),
            "wo": np.ascontiguousarray(Wo[es, :].astype(BF)),
            "bqk": np.ascontiguousarray(bqk),
            "bv": bv[es].reshape(1, EL).copy(),
            "bo": np.ascontiguousarray((bo * 0.5).reshape(1, D)),
            "gamma": gamma.reshape(1, D).copy(),
            "beta": beta.reshape(1, D).copy(),
        })

    global _LAST_IN_MAPS
    _LAST_IN_MAPS = in_maps
    res = run_bass_kernel_spmd(nc, in_maps, core_ids=list(range(8)))

    out = np.empty((B, S, D), dtype=np.float32)
    for c in range(8):
        b, hf = c // 2, c % 2
        out[b, 1024 * hf:1024 * (hf + 1)] = res.results[c]["y"][1024 * hf:1024 * (hf + 1)]
    return out
